# revision 8
# baseline (speedup 1.0000x reference)
"""Contrastive loss (CLIP-style BCE) on 8 Trainium2 NeuronCores.

Strategy: data-parallel over the batch dim. Each core gets a 128-row shard of
img_features plus the full (replicated) text_embeds/labels, computes its
[128, 1024] slice of the logits and a partial sum of
softplus(logits) - logits * targets; the host sums the 8 partial scalars and
divides by B*B.

The dominant cost is streaming the img_features shard (75.5 MB/core) for the
H*W pooling; everything downstream is tiny and hides under the DMA.
"""

import numpy as np

import concourse.bacc as bacc
import concourse.mybir as mybir
import concourse.tile as tile
from concourse.bass_utils import run_bass_kernel_spmd
from concourse.masks import make_identity

N_CORES = 8
B, C, H, W = 1024, 256, 24, 24
HW = H * W  # 576
BS = B // N_CORES  # 128 rows per core
P = 128
TEMP = 0.07
INV_TEMP = 1.0 / TEMP
NB = 4  # batches per pooling DMA (1.15 MB per transfer)

F32 = mybir.dt.float32
ALU = mybir.AluOpType
ACT = mybir.ActivationFunctionType
AX = mybir.AxisListType

_NC_CACHE = []


def _emit_rsqrt(nc, small, ns, out_rv, tag):
    """out_rv = 1/sqrt(ns), via exp(-0.5*ln(ns)) + one Newton step.

    Avoids the Sqrt activation table (kernel stays on the
    natural_log_exp_and_others set) and the banned Rsqrt activation.
    """
    F32 = mybir.dt.float32
    y0 = small.tile([P, 1], F32, tag=f"{tag}_y0")
    nc.scalar.activation(y0, ns, ACT.Ln)
    nc.scalar.activation(y0, y0, ACT.Exp, scale=-0.5)
    # Newton: y1 = y0 * (1.5 - 0.5 * ns * y0^2)
    t1 = small.tile([P, 1], F32, tag=f"{tag}_t1")
    nc.vector.tensor_mul(t1, y0, y0)
    nc.vector.tensor_mul(t1, t1, ns)
    nc.vector.tensor_scalar(
        out=t1, in0=t1, scalar1=-0.5, scalar2=1.5, op0=ALU.mult, op1=ALU.add
    )
    nc.vector.tensor_mul(out_rv, y0, t1)


def _build_nc():
    nc = bacc.Bacc("TRN2", target_bir_lowering=False, debug=False, num_devices=N_CORES)
    img = nc.dram_tensor("img", [BS, C, HW], F32, kind="ExternalInput").ap()
    txt = nc.dram_tensor("txt", [B, C], F32, kind="ExternalInput").ap()
    lab_row = nc.dram_tensor("lab_row", [BS, 1], F32, kind="ExternalInput").ap()
    lab_all = nc.dram_tensor("lab_all", [1, B], F32, kind="ExternalInput").ap()
    out = nc.dram_tensor("partial", [1, 1], F32, kind="ExternalOutput").ap()

    with tile.TileContext(nc) as tc:
        with (
            tc.tile_pool(name="consts", bufs=1) as consts,
            tc.tile_pool(name="big", bufs=6) as big,
            tc.tile_pool(name="txtp", bufs=3) as txtp,
            tc.tile_pool(name="small", bufs=4) as small,
            tc.tile_pool(name="persist", bufs=1) as persist,
            tc.tile_pool(name="psum_tp", bufs=2, space="PSUM") as psum_tp,
            tc.tile_pool(name="psum_g", bufs=2, space="PSUM") as psum_g,
        ):
            identity = consts.tile([P, P], F32, tag="identity")
            make_identity(nc, identity)
            ones = consts.tile([P, 1], F32, tag="ones")
            nc.vector.memset(ones, 1.0)

            # ---- pooling: pooledT[cb][c, b] = sum_hw img[b, cb*128+c, hw] ----
            # (mean/576 is skipped: l2-normalization cancels positive scaling)
            pooledT = [
                persist.tile([P, BS], F32, tag=f"pooledT{cb}", name=f"pooledT{cb}")
                for cb in range(2)
            ]
            for b0 in range(0, BS, NB):
                for cb in range(2):
                    it = big.tile([P, NB, HW], F32, tag="imgin")
                    src = img[b0 : b0 + NB, cb * P : (cb + 1) * P, :].rearrange(
                        "b c h -> c b h"
                    )
                    nc.sync.dma_start(out=it, in_=src)
                    nc.vector.reduce_sum(
                        out=pooledT[cb][:, b0 : b0 + NB], in_=it, axis=AX.X
                    )

            # ---- text: row-normalize then transpose to [C, B] layout ----
            txtT = [
                persist.tile([P, B], F32, tag=f"txtT{cb}", name=f"txtT{cb}")
                for cb in range(2)
            ]
            for tb in range(B // P):
                tt = txtp.tile([P, C], F32, tag="ttin")
                nc.sync.dma_start(out=tt, in_=txt[tb * P : (tb + 1) * P, :])
                tns = small.tile([P, 1], F32, tag="tns")
                sq_scr = small.tile([P, C], F32, tag="sq_scr")
                nc.scalar.activation(sq_scr, tt, ACT.Square, accum_out=tns)
                trv = small.tile([P, 1], F32, tag="trv")
                _emit_rsqrt(nc, small, tns, trv, "trsq")
                tn = txtp.tile([P, C], F32, tag="ttn")
                nc.vector.tensor_scalar_mul(tn, tt, trv)
                for cb in range(2):
                    pt = psum_tp.tile([P, P], F32, tag="pt")
                    # PE is_transpose crashes this runtime; a regular matmul
                    # against identity computes the exact transpose instead
                    nc.tensor.matmul(
                        pt, tn[:, cb * P : (cb + 1) * P], identity, start=True, stop=True
                    )
                    nc.scalar.copy(txtT[cb][:, tb * P : (tb + 1) * P], pt)

            # ---- targets[p, j] = (lab_row[p] == lab_all[j]) via relu(1-d^2) ----
            lab_row_sb = consts.tile([P, 1], F32, tag="lab_row_sb")
            nc.sync.dma_start(out=lab_row_sb, in_=lab_row)
            tgt = persist.tile([P, B], F32, tag="tgt")
            nc.sync.dma_start(out=tgt, in_=lab_all.to_broadcast([P, B]))
            nc.vector.tensor_scalar_sub(tgt, tgt, lab_row_sb)
            nc.scalar.activation(tgt, tgt, ACT.Square)
            nc.scalar.activation(tgt, tgt, ACT.Relu, scale=-1.0, bias=1.0)
            nc.scalar.mul(tgt, tgt, INV_TEMP)  # fold 1/T into the 0/1 mask

            # ---- pooled row norms (needs [b, c] layout: transpose back) ----
            pooled = persist.tile([P, C], F32, tag="pooled")
            for cb in range(2):
                pp = psum_tp.tile([P, P], F32, tag="pt")
                nc.tensor.matmul(pp, pooledT[cb][:, :], identity, start=True, stop=True)
                nc.scalar.copy(pooled[:, cb * P : (cb + 1) * P], pp)
            ns = small.tile([P, 1], F32, tag="ns")
            psq_scr = small.tile([P, C], F32, tag="sq_scr")
            nc.scalar.activation(psq_scr, pooled, ACT.Square, accum_out=ns)
            rv = small.tile([P, 1], F32, tag="rv")
            _emit_rsqrt(nc, small, ns, rv, "prsq")
            pooled_n = persist.tile([P, C], F32, tag="pooled_n")
            nc.vector.tensor_scalar_mul(pooled_n, pooled, rv)
            pnT = [
                persist.tile([P, P], F32, tag=f"pnT{cb}", name=f"pnT{cb}")
                for cb in range(2)
            ]
            for cb in range(2):
                pq = psum_tp.tile([P, P], F32, tag="pt")
                nc.tensor.matmul(
                    pq, pooled_n[:, cb * P : (cb + 1) * P], identity, start=True, stop=True
                )
                nc.scalar.copy(pnT[cb][:], pq)

            # ---- gram [128, 1024] + softplus/target accumulation ----
            sp_acc = small.tile([P, 2], F32, tag="sp_acc")
            xt_acc = small.tile([P, 2], F32, tag="xt_acc")
            for nb in range(2):
                g = psum_g.tile([P, 512], F32, tag="g")
                for cb in range(2):
                    nc.tensor.matmul(
                        g,
                        pnT[cb][:],
                        txtT[cb][:, nb * 512 : (nb + 1) * 512],
                        start=(cb == 0),
                        stop=(cb == 1),
                    )
                # softplus(x) = ln(exp(x) + 1); |x| <= 1/0.07 so exp can't overflow
                e_scr = small.tile([P, 512], F32, tag="e_scr")
                nc.scalar.activation(e_scr, g, ACT.Exp, scale=INV_TEMP)
                sp_scr = small.tile([P, 512], F32, tag="sp_scr")
                nc.scalar.activation(
                    sp_scr,
                    e_scr,
                    ACT.Ln,
                    bias=1.0,
                    accum_out=sp_acc[:, nb : nb + 1],
                )
                xt_scr = small.tile([P, 512], F32, tag="xt_scr")
                nc.vector.tensor_mul(xt_scr, g, tgt[:, nb * 512 : (nb + 1) * 512])
                nc.vector.reduce_sum(
                    out=xt_acc[:, nb : nb + 1], in_=xt_scr, axis=AX.X
                )

            # ---- total per partition, then 128-way reduce via matmul ----
            tot = small.tile([P, 1], F32, tag="tot")
            nc.vector.reduce_sum(out=tot, in_=sp_acc, axis=AX.X)
            xtt = small.tile([P, 1], F32, tag="xtt")
            nc.vector.reduce_sum(out=xtt, in_=xt_acc, axis=AX.X)
            nc.vector.tensor_sub(tot, tot, xtt)
            ps = psum_tp.tile([1, 1], F32, tag="ps")
            nc.tensor.matmul(ps, tot, ones, start=True, stop=True)
            res = small.tile([1, 1], F32, tag="res")
            nc.scalar.copy(res, ps)
            nc.sync.dma_start(out=out, in_=res)

    nc.finalize()
    return nc


def _get_nc():
    if not _NC_CACHE:
        _NC_CACHE.append(_build_nc())
    return _NC_CACHE[0]


def kernel(img_features, text_embeds, labels):
    img_features = np.ascontiguousarray(np.asarray(img_features, dtype=np.float32))
    text_embeds = np.ascontiguousarray(np.asarray(text_embeds, dtype=np.float32))
    labels_f = np.asarray(labels).astype(np.float32)  # values < 16: exact in f32

    img3 = img_features.reshape(B, C, HW)
    nc = _get_nc()
    in_maps = []
    for i in range(N_CORES):
        sl = slice(i * BS, (i + 1) * BS)
        in_maps.append(
            {
                "img": img3[sl],
                "txt": text_embeds,
                "lab_row": labels_f[sl].reshape(BS, 1),
                "lab_all": labels_f.reshape(1, B),
            }
        )
    r = run_bass_kernel_spmd(nc, in_maps, core_ids=list(range(N_CORES)))
    total = sum(float(r.results[i]["partial"][0, 0]) for i in range(N_CORES))
    return np.float32(total / (B * B))


# revision 12
# speedup vs baseline: 1.2614x; 1.2614x over previous
"""Contrastive loss (CLIP-style BCE) on 8 Trainium2 NeuronCores.

Strategy: data-parallel over the batch dim. Each core gets a 128-row shard of
img_features plus the full (replicated) text_embeds/labels, computes its
[128, 1024] slice of the logits and a partial sum of
softplus(logits) - logits * targets; the host sums the 8 partial scalars and
divides by B*B.

The dominant cost is streaming the img_features shard (75.5 MB/core) for the
H*W pooling; everything downstream is tiny and hides under the DMA.

Runtime notes (found by bisection on this axon/fakenrt stack):
- PE is_transpose matmuls and InstTensorTensorReduce crash the exec unit;
  transposes are done as regular matmuls against identity, reductions via
  ACT accum_out / DVE reduce_sum.
- Softplus/Sqrt have no activation tables here; softplus = ln(exp(x)+1)
  (|x| <= 1/0.07 so exp is safe), rsqrt = exp(-0.5*ln(x)) + one Newton step.
"""

import numpy as np

import concourse.bacc as bacc
import concourse.mybir as mybir
import concourse.tile as tile
from concourse.bass_utils import run_bass_kernel_spmd
from concourse.masks import make_identity

N_CORES = 8
B, C, H, W = 1024, 256, 24, 24
HW = H * W  # 576
BS = B // N_CORES  # 128 rows per core
P = 128
TEMP = 0.07
INV_TEMP = 1.0 / TEMP
NB = 4  # batches per pooling DMA (1.15 MB per transfer)

F32 = mybir.dt.float32
ALU = mybir.AluOpType
ACT = mybir.ActivationFunctionType
AX = mybir.AxisListType

_NC_CACHE = []


def _emit_rsqrt(nc, small, ns, out_rv, tag):
    """out_rv = 1/sqrt(ns), via exp(-0.5*ln(ns)) + one Newton step.

    Avoids the Sqrt activation table (kernel stays on the
    natural_log_exp_and_others set) and the banned Rsqrt activation.
    """
    y0 = small.tile([P, 1], F32, tag=f"{tag}_y0", name=f"{tag}_y0")
    nc.scalar.activation(y0, ns, ACT.Ln)
    nc.scalar.activation(y0, y0, ACT.Exp, scale=-0.5)
    # Newton: y1 = y0 * (1.5 - 0.5 * ns * y0^2)
    t1 = small.tile([P, 1], F32, tag=f"{tag}_t1", name=f"{tag}_t1")
    nc.vector.tensor_mul(t1, y0, y0)
    nc.vector.tensor_mul(t1, t1, ns)
    nc.vector.tensor_scalar(
        out=t1, in0=t1, scalar1=-0.5, scalar2=1.5, op0=ALU.mult, op1=ALU.add
    )
    nc.vector.tensor_mul(out_rv, y0, t1)


def _emit_body(nc, pools, identity, ones, img, txt, lab_row, lab_all, out, cfg):
    consts, big, txtp, small, persist, psum_tp, psum_g = pools
    nb_sz = cfg.get("nb", NB)
    dma = nc.gpsimd if cfg.get("swdge") else nc.sync

    # ---- pooling: pooledT[cb][c, b] = sum_hw img[b, cb*128+c, hw] ----
    # (mean/576 is skipped: l2-normalization cancels positive scaling)
    pooledT = [
        persist.tile([P, BS], F32, tag=f"pooledT{cb}", name=f"pooledT{cb}")
        for cb in range(2)
    ]
    if cfg.get("dma_only"):
        for pt_ in pooledT:
            nc.vector.memset(pt_, 1.0)
    it0 = None
    ascr = None
    tile_idx = -1
    for b0 in range(0, BS, nb_sz):
        for cb in range(2):
            tile_idx += 1
            if cfg.get("reduce_only"):
                if it0 is None:
                    it0 = big.tile([P, nb_sz, HW], F32, tag="imgin", name="imgin")
                    src = img[0:nb_sz, 0:P, :].rearrange("b c h -> c b h")
                    dma.dma_start(out=it0, in_=src)
                nc.vector.reduce_sum(
                    out=pooledT[cb][:, b0 : b0 + nb_sz], in_=it0, axis=AX.X
                )
                continue
            it = big.tile([P, nb_sz, HW], F32, tag="imgin", name="imgin")
            src = img[b0 : b0 + nb_sz, cb * P : (cb + 1) * P, :].rearrange(
                "b c h -> c b h"
            )
            # two physical HWDGE rings (SP + ACT): alternate to double DMA issue
            d_eng = (nc.sync if cb == 0 else nc.scalar) if cfg.get("dma2") else dma
            d_eng.dma_start(out=it, in_=src)
            if cfg.get("dma_only"):
                continue
            if cfg.get("redsplit") and tile_idx % 4 != 0:
                # ACT reduce: one Identity+accum per batch row
                for j in range(nb_sz):
                    if ascr is None:
                        ascr = big.tile([P, HW], F32, tag="ascr", name="ascr")
                    nc.scalar.activation(
                        ascr,
                        it[:, j, :],
                        ACT.Identity,
                        accum_out=pooledT[cb][:, b0 + j : b0 + j + 1],
                    )
            elif cfg.get("red2d"):
                for j in range(nb_sz):
                    nc.vector.reduce_sum(
                        out=pooledT[cb][:, b0 + j : b0 + j + 1],
                        in_=it[:, j, :],
                        axis=AX.X,
                    )
            else:
                nc.vector.reduce_sum(
                    out=pooledT[cb][:, b0 : b0 + nb_sz], in_=it, axis=AX.X
                )

    # ---- text: row-normalize then transpose to [C, B] layout ----
    txtT = [
        persist.tile([P, B], F32, tag=f"txtT{cb}", name=f"txtT{cb}") for cb in range(2)
    ]
    for tb in range(B // P):
        tt = txtp.tile([P, C], F32, tag="ttin", name="ttin")
        nc.sync.dma_start(out=tt, in_=txt[tb * P : (tb + 1) * P, :])
        tns = small.tile([P, 1], F32, tag="tns", name="tns")
        sq_scr = small.tile([P, C], F32, tag="sq_scr", name="sq_scr")
        nc.scalar.activation(sq_scr, tt, ACT.Square, accum_out=tns)
        trv = small.tile([P, 1], F32, tag="trv", name="trv")
        _emit_rsqrt(nc, small, tns, trv, "trsq")
        tn = txtp.tile([P, C], F32, tag="ttn", name="ttn")
        nc.vector.tensor_scalar_mul(tn, tt, trv)
        for cb in range(2):
            pt = psum_tp.tile([P, P], F32, tag="pt", name="pt")
            # PE is_transpose crashes this runtime; a regular matmul against
            # identity computes the exact transpose instead
            nc.tensor.matmul(
                pt, tn[:, cb * P : (cb + 1) * P], identity, start=True, stop=True
            )
            nc.scalar.copy(txtT[cb][:, tb * P : (tb + 1) * P], pt)

    # ---- targets[p, j] = (lab_row[p] == lab_all[j]) via relu(1-d^2) ----
    lab_row_sb = small.tile([P, 1], F32, tag="lab_row_sb", name="lab_row_sb")
    nc.sync.dma_start(out=lab_row_sb, in_=lab_row)
    tgt = persist.tile([P, B], F32, tag="tgt", name="tgt")
    nc.sync.dma_start(out=tgt, in_=lab_all.to_broadcast([P, B]))
    nc.vector.tensor_scalar_sub(tgt, tgt, lab_row_sb)
    nc.scalar.activation(tgt, tgt, ACT.Square)
    nc.scalar.activation(tgt, tgt, ACT.Relu, scale=-1.0, bias=1.0)
    nc.scalar.mul(tgt, tgt, INV_TEMP)  # fold 1/T into the 0/1 mask

    # ---- pooled row norms (needs [b, c] layout: transpose back) ----
    pooled = persist.tile([P, C], F32, tag="pooled", name="pooled")
    for cb in range(2):
        pp = psum_tp.tile([P, P], F32, tag="pt", name="pt")
        nc.tensor.matmul(pp, pooledT[cb][:, :], identity, start=True, stop=True)
        nc.scalar.copy(pooled[:, cb * P : (cb + 1) * P], pp)
    ns = small.tile([P, 1], F32, tag="ns", name="ns")
    psq_scr = small.tile([P, C], F32, tag="sq_scr", name="psq_scr")
    nc.scalar.activation(psq_scr, pooled, ACT.Square, accum_out=ns)
    rv = small.tile([P, 1], F32, tag="rv", name="rv")
    _emit_rsqrt(nc, small, ns, rv, "prsq")
    pooled_n = persist.tile([P, C], F32, tag="pooled_n", name="pooled_n")
    nc.vector.tensor_scalar_mul(pooled_n, pooled, rv)
    pnT = [persist.tile([P, P], F32, tag=f"pnT{cb}", name=f"pnT{cb}") for cb in range(2)]
    for cb in range(2):
        pq = psum_tp.tile([P, P], F32, tag="pt", name="pt")
        nc.tensor.matmul(
            pq, pooled_n[:, cb * P : (cb + 1) * P], identity, start=True, stop=True
        )
        nc.scalar.copy(pnT[cb][:], pq)

    # ---- gram [128, 1024] + softplus/target accumulation ----
    sp_acc = small.tile([P, 2], F32, tag="sp_acc", name="sp_acc")
    xt_acc = small.tile([P, 2], F32, tag="xt_acc", name="xt_acc")
    for nbk in range(2):
        g = psum_g.tile([P, 512], F32, tag="g", name="g")
        for cb in range(2):
            nc.tensor.matmul(
                g,
                pnT[cb][:],
                txtT[cb][:, nbk * 512 : (nbk + 1) * 512],
                start=(cb == 0),
                stop=(cb == 1),
            )
        # softplus(x) = ln(exp(x) + 1); |x| <= 1/0.07 so exp can't overflow
        e_scr = small.tile([P, 512], F32, tag="e_scr", name="e_scr")
        nc.scalar.activation(e_scr, g, ACT.Exp, scale=INV_TEMP)
        sp_scr = small.tile([P, 512], F32, tag="sp_scr", name="sp_scr")
        nc.scalar.activation(
            sp_scr, e_scr, ACT.Ln, bias=1.0, accum_out=sp_acc[:, nbk : nbk + 1]
        )
        xt_scr = small.tile([P, 512], F32, tag="xt_scr", name="xt_scr")
        nc.vector.tensor_mul(xt_scr, g, tgt[:, nbk * 512 : (nbk + 1) * 512])
        nc.vector.reduce_sum(out=xt_acc[:, nbk : nbk + 1], in_=xt_scr, axis=AX.X)

    # ---- total per partition, then 128-way reduce via matmul ----
    tot = small.tile([P, 1], F32, tag="tot", name="tot")
    nc.vector.reduce_sum(out=tot, in_=sp_acc, axis=AX.X)
    xtt = small.tile([P, 1], F32, tag="xtt", name="xtt")
    nc.vector.reduce_sum(out=xtt, in_=xt_acc, axis=AX.X)
    nc.vector.tensor_sub(tot, tot, xtt)
    ps = psum_tp.tile([1, 1], F32, tag="ps", name="ps")
    nc.tensor.matmul(ps, tot, ones, start=True, stop=True)
    res = small.tile([1, 1], F32, tag="res", name="res")
    nc.scalar.copy(res, ps)
    nc.sync.dma_start(out=out, in_=res)


def _build_nc(reps=1, **cfg):
    # production defaults (benched): per-batch 2D DVE reduces beat one 3D
    # reduce ~2.5x per element, keeping DVE well under the DMA plateau;
    # 8 in-flight pooling tiles deepen the DMA pipeline
    cfg.setdefault("red2d", True)
    cfg.setdefault("big_bufs", 8)
    nc = bacc.Bacc("TRN2", target_bir_lowering=False, debug=False, num_devices=N_CORES)
    img = nc.dram_tensor("img", [BS, C, HW], F32, kind="ExternalInput").ap()
    txt = nc.dram_tensor("txt", [B, C], F32, kind="ExternalInput").ap()
    lab_row = nc.dram_tensor("lab_row", [BS, 1], F32, kind="ExternalInput").ap()
    lab_all = nc.dram_tensor("lab_all", [1, B], F32, kind="ExternalInput").ap()
    outs = [
        nc.dram_tensor(
            "partial" if r == 0 else f"partial{r}", [1, 1], F32, kind="ExternalOutput"
        ).ap()
        for r in range(reps)
    ]

    with tile.TileContext(nc) as tc:
        with (
            tc.tile_pool(name="consts", bufs=1) as consts,
            tc.tile_pool(name="big", bufs=cfg.get("big_bufs", 6)) as big,
            tc.tile_pool(name="txtp", bufs=3) as txtp,
            tc.tile_pool(name="small", bufs=4) as small,
            tc.tile_pool(name="persist", bufs=cfg.get("persist_bufs", 1)) as persist,
            tc.tile_pool(name="psum_tp", bufs=2, space="PSUM") as psum_tp,
            tc.tile_pool(name="psum_g", bufs=2, space="PSUM") as psum_g,
        ):
            identity = consts.tile([P, P], F32, tag="identity")
            make_identity(nc, identity)
            ones = consts.tile([P, 1], F32, tag="ones")
            nc.vector.memset(ones, 1.0)
            pools = (consts, big, txtp, small, persist, psum_tp, psum_g)
            for r in range(reps):
                _emit_body(
                    nc, pools, identity, ones, img, txt, lab_row, lab_all, outs[r], cfg
                )

    nc.finalize()
    return nc


def _get_nc():
    if not _NC_CACHE:
        _NC_CACHE.append(_build_nc())
    return _NC_CACHE[0]


def kernel(img_features, text_embeds, labels):
    img_features = np.ascontiguousarray(np.asarray(img_features, dtype=np.float32))
    text_embeds = np.ascontiguousarray(np.asarray(text_embeds, dtype=np.float32))
    labels_f = np.asarray(labels).astype(np.float32)  # values < 16: exact in f32

    img3 = img_features.reshape(B, C, HW)
    nc = _get_nc()
    in_maps = []
    for i in range(N_CORES):
        sl = slice(i * BS, (i + 1) * BS)
        in_maps.append(
            {
                "img": img3[sl],
                "txt": text_embeds,
                "lab_row": labels_f[sl].reshape(BS, 1),
                "lab_all": labels_f.reshape(1, B),
            }
        )
    r = run_bass_kernel_spmd(nc, in_maps, core_ids=list(range(N_CORES)))
    total = sum(float(r.results[i]["partial"][0, 0]) for i in range(N_CORES))
    return np.float32(total / (B * B))
